# revision 26
# baseline (speedup 1.0000x reference)
"""Trainium2 Bass kernel for nn_AutoencoderHybrid_65481071408310.

Math: the reference simulates an 8-qubit circuit per sample. The RX-encoding
layer produces a product state whose amplitudes factor as
    psi[k] = m[k] * (-i)^popcount(k),   m[k] = prod_i (cos(x_i/2) or sin(x_i/2))
and the StronglyEntanglingLayers form a fixed 256x256 unitary U that depends
only on q_weights.  Folding the popcount phases into U gives a REAL matmul
    phi = m @ V,  V = [Re(W) | Im(W)],  W = (U * (-i)^popcount)^T   (256 x 512)
then probs256 = Re^2 + Im^2 (paired), z_i = probs @ signs, and the MLP head.
signs@w1.T folds into A (256x4), contracted against probs256 with K=256.

Device pipeline per core (batch 8192, fp16 matmul operands), software
pipelined in 4 phases of 2048 samples with staging issued TWO phases ahead:
  front: cos/sin (ACT) -> 8 PE transposes -> csf_all (wire, sample) gather
  stage (per phase): pairsA replication DMA -> pairs mul (DVE) ->
    hiloA/hiloB DMAs -> hilo mul -> mtA0/mtA1/mtB DMAs -> mt0/mt1 muls
  compute (per 512-sample block): 8 phi matmuls (K=256 -> 512 wide, PSUM),
    ACT Square -> f16, DVE pair-add -> probs256, 2 A-matmuls -> preh,
    relu(+b1) ACT/DVE alternating, 4 w2 matmuls, +b2 on DVE copy-out.
"""
import sys
import numpy as np

sys.path.insert(0, '/opt/trn_rl_repo')

import concourse.bacc as bacc
import concourse.mybir as mybir
import concourse.tile as tile
from concourse.bass_utils import run_bass_kernel_spmd

F32 = mybir.dt.float32
F16 = mybir.dt.float16
AFT = mybir.ActivationFunctionType
ALU = mybir.AluOpType

NQ = 8
DIM = 256
REPS = 4
INPUT_DIM = 8
LATENT = 4
BATCH = 65536
NCORES = 8
BC = BATCH // NCORES          # 8192 samples per core
NPHASE = 8
NCH = 8                       # 128-sample chunks per phase
CH = NCH * 128                # 2048 samples per phase
NBLK = CH // 512              # 4 blocks of 512 samples per phase
NWARM = 24                    # PE clock-ramp warmup matmuls

LAST_RESULTS = None           # test harness introspection


# ---------------------------------------------------------------- host math
def _rot_mat(phi, theta, omega):
    c, s = np.cos(theta / 2), np.sin(theta / 2)
    return np.array([
        [np.exp(-0.5j * (phi + omega)) * c, -np.exp(0.5j * (phi - omega)) * s],
        [np.exp(-0.5j * (phi - omega)) * s, np.exp(0.5j * (phi + omega)) * c],
    ], dtype=np.complex128)


def _kron_list(ops):
    full = ops[0]
    for o in ops[1:]:
        full = np.kron(full, o)
    return full


def _build_entangler(qw):
    I2 = np.eye(2, dtype=np.complex128)
    P0 = np.array([[1, 0], [0, 0]], dtype=np.complex128)
    P1 = np.array([[0, 0], [0, 1]], dtype=np.complex128)
    X = np.array([[0, 1], [1, 0]], dtype=np.complex128)
    U = np.eye(DIM, dtype=np.complex128)
    for l in range(REPS):
        for i in range(NQ):
            ops = [I2] * NQ
            ops[i] = _rot_mat(*qw[l, i])
            U = _kron_list(ops) @ U
        r = (l % (NQ - 1)) + 1
        for i in range(NQ):
            t = (i + r) % NQ
            ops0 = [I2] * NQ
            ops0[i] = P0
            ops1 = [I2] * NQ
            ops1[i] = P1
            ops1[t] = X
            U = (_kron_list(ops0) + _kron_list(ops1)) @ U
    return U


def _host_consts(q_weights, w1, b1, w2, b2):
    U = _build_entangler(q_weights.astype(np.float64))
    pop = np.array([bin(k).count('1') for k in range(DIM)])
    W = (U * ((-1j) ** pop)[None, :]).T          # phi = m @ W
    V = np.concatenate([W.real, W.imag], axis=1)  # (256, 512)
    ks = np.arange(DIM)
    signs = 1.0 - 2.0 * ((ks[:, None] >> (NQ - 1 - np.arange(NQ))[None, :]) & 1)
    A = signs @ w1.T.astype(np.float64)           # (256, 4)
    vmat = V.reshape(2, 128, 512).transpose(1, 0, 2).reshape(128, 1024)
    amat = A.reshape(2, 128, LATENT).transpose(1, 0, 2).reshape(128, 2 * LATENT)
    # f16 blob: [vmat 0:1024 | amat 1024:1032 | w2.T rows0:4 1032:1040 |
    #            ident 1040:1168 | selA0 1168:1296 | selA1 1296:1424]
    blob16 = np.zeros((128, 1424), np.float16)
    blob16[:, 0:1024] = vmat.astype(np.float16)
    blob16[:, 1024:1032] = amat.astype(np.float16)
    blob16[0:LATENT, 1032:1040] = w2.T.astype(np.float16)
    blob16[:, 1040:1168] = np.eye(128, dtype=np.float16)
    ks = np.arange(128)
    selA0 = (ks[None, :] // 16 == np.arange(16)[:, None]).astype(np.float16)
    selA1 = (8 + ks[None, :] // 16 == np.arange(16)[:, None]).astype(np.float16)
    blob16[0:16, 1168:1296] = selA0
    blob16[0:16, 1296:1424] = selA1
    # f32 blob: [b2 tiled 0:32 | b1 col 32]
    blob32 = np.zeros((128, 33), np.float32)
    blob32[:, 0:32] = np.tile(b2.astype(np.float32), 4)[None, :]
    blob32[0:LATENT, 32] = b1.astype(np.float32)
    return {'blob16': np.ascontiguousarray(blob16),
            'blob32': np.ascontiguousarray(blob32)}


# ---------------------------------------------------------------- bass build
def _build_nc():
    nc = bacc.Bacc(None, target_bir_lowering=False)
    xs = nc.declare_dram_parameter("xs", [BC, INPUT_DIM], F32, isOutput=False)
    blob16 = nc.declare_dram_parameter("blob16", [128, 1424], F16, isOutput=False)
    blob32 = nc.declare_dram_parameter("blob32", [128, 33], F32, isOutput=False)
    out = nc.declare_dram_parameter("out", [BC, INPUT_DIM], F32, isOutput=True)

    with tile.TileContext(nc) as tc:
        with (
            tc.tile_pool(name="const", bufs=1) as cst,
            tc.tile_pool(name="front", bufs=1) as frt,
            tc.tile_pool(name="stage", bufs=3) as stg,
            tc.tile_pool(name="mtp", bufs=3) as mtp,
            tc.tile_pool(name="sqp", bufs=2) as sqp,
            tc.tile_pool(name="prp", bufs=2) as prp,
            tc.tile_pool(name="sml", bufs=2) as sml,
        ):
            # ---- input load first (critical path)
            xnat = frt.tile([128, BC // 16], F32)      # free = (n:64, d:8)
            nc.sync.dma_start(xnat[:], xs.rearrange("(p n) d -> p n d", n=64))
            # ---- constants (2 packed DMAs on otherwise-idle queues)
            c16 = cst.tile([128, 1424], F16)
            nc.scalar.dma_start(c16[:], blob16[:])
            c32 = cst.tile([128, 33], F32)
            nc.gpsimd.dma_start(c32[:], blob32[:])
            vt = c16[:, 0:1024]
            at = c16[:, 1024:1032]
            w2s = c16[0:LATENT, 1032:1040]
            ids = c16[:, 1040:1168]
            selA0 = c16[0:16, 1168:1296]
            selA1 = c16[0:16, 1296:1424]
            b2s = c32[:, 0:32]
            b1s = c32[0:LATENT, 32:33]
            halfpi = cst.tile([128, 1], F32)
            nc.vector.memset(halfpi[:], float(np.pi / 2))
            # ---- whole-core cos/sin, free = (d, n); sample = 64p + n
            # (warm primes the Sin table with no data dependencies)
            warm = cst.tile([1, 1], F16)
            nc.scalar.activation(warm[:], halfpi[0:1, :], AFT.Sin, scale=0.0,
                                 bias=0.0)
            cnat = frt.tile([128, BC // 16], F16)
            snat = frt.tile([128, BC // 16], F16)
            xdn = xnat.rearrange("p (n d) -> p d n", d=8)

            # ---- 8 transposes into csT: row 64*(w%2)+m,
            #      free col = 512*t + 128*(w//2) + p   (t: 0=cos 1=sin)
            csT = frt.tile([128, 1024], F16)
            # csf_all, free col = 128*m + p  -> sample 64*p + m
            #   rows 0:8  = even wires: row 2*q+tA          = cs_tA(wire 2q)
            #   rows 32:48 = odd, interleaved: 32+4q+2tA+tB = cs_tB(wire 2q+1)
            csf_all = frt.tile([48, BC], F16)
            pools = {}

            def csf_gather():
                engs = [nc.gpsimd, nc.sync, nc.scalar]
                i = 0
                for q in range(4):
                    for tA in range(2):
                        for tB in range(2):
                            src = csT[64:128, 512 * tB + 128 * q:
                                      512 * tB + 128 * (q + 1)]
                            r = 32 + 4 * q + 2 * tA + tB
                            dst = csf_all[r:r + 1, :]
                            engs[i % 3].dma_start(dst, src)
                            i += 1
                for q in range(4):
                    for tA in range(2):
                        src = csT[0:64, 512 * tA + 128 * q:
                                  512 * tA + 128 * (q + 1)]
                        dst = csf_all[2 * q + tA:2 * q + tA + 1, :]
                        engs[i % 3].dma_start(dst, src)
                        i += 1

            def stage_q(ph):
                """pairsA replication DMA for phase ph (issued 2 ahead)."""
                sl = slice(CH * ph, CH * (ph + 1))
                pairsA = stg.tile([48, CH], F16, tag="pairsA")
                nc.gpsimd.dma_start(
                    pairsA[32:48, :],
                    csf_all[0:8, sl].unsqueeze(1).broadcast_to([8, 2, CH]))
                return (pairsA, sl)

            def stage_mul1(ph, pairsA, sl):
                early = ph <= 2
                pairs = stg.tile([48, CH], F16, tag="pairs")
                nc.vector.tensor_mul(pairs[32:48, :], pairsA[32:48, :],
                                     csf_all[32:48, sl])
                hiloA = stg.tile([32, CH], F16, tag="hiloA")
                hiloB = stg.tile([32, CH], F16, tag="hiloB")
                if early:
                    engsA = [nc.gpsimd, nc.sync]
                    engsB = [[nc.sync, nc.scalar, nc.gpsimd, nc.sync],
                             [nc.scalar, nc.gpsimd, nc.sync, nc.scalar]]
                else:
                    engsA = [nc.scalar, nc.scalar]
                    engsB = [[nc.gpsimd] * 4, [nc.sync] * 4]
                for h in range(2):
                    engsA[h].dma_start(
                        hiloA[16 * h:16 * (h + 1), :],
                        pairs[32 + 8 * h:32 + 8 * h + 4, :].unsqueeze(1)
                        .broadcast_to([4, 4, CH]))
                    for r in range(4):
                        engsB[h][r].dma_start(
                            hiloB[16 * h + 4 * r:16 * h + 4 * (r + 1), :],
                            pairs[32 + 8 * h + 4:32 + 8 * h + 8, :])
                return pairs, hiloA, hiloB

            def stage_mul2(ph, hiloA, hiloB):
                hilo = stg.tile([32, CH], F16, tag="hilo")
                nc.vector.tensor_mul(hilo[:], hiloA[:], hiloB[:])
                # mtB: mlo tiled x8; two direct hops then parallel doubling
                mtB = stg.tile([128, CH], F16, tag="mtB")
                nc.sync.dma_start(mtB[0:16, :], hilo[16:32, :])
                nc.gpsimd.dma_start(mtB[16:32, :], hilo[16:32, :])
                nc.sync.dma_start(mtB[32:64, :], mtB[0:32, :])
                nc.gpsimd.dma_start(mtB[64:96, :], mtB[0:32, :])
                nc.sync.dma_start(mtB[96:128, :], mtB[0:32, :])
                return hilo, mtB

            def sel_mtA(hilo, sel):
                # PE row-replication: mtA[p, n] = hilo[sel-row(p), n]
                # (two matmuls: each output must fit one PSUM bank)
                mtA = pools['mtap'].tile([128, CH], F32, tag="mtA")
                for e in range(2):
                    nc.tensor.matmul(mtA[:, 512 * e:512 * (e + 1)], sel,
                                     hilo[0:16, 512 * e:512 * (e + 1)],
                                     start=True, stop=True)
                return mtA

            def stage_mt(mtA, mtB, tag):
                mt = mtp.tile([128, CH], F16, tag=tag)
                nc.vector.tensor_mul(mt[:], mtA[:], mtB[:])
                return mt

            def block_phi(g, mt0, mt1):
                phip = pools['phip']
                sl = slice(512 * g, 512 * (g + 1))
                sqs = []
                for jp in range(2):
                    phi = phip.tile([128, 1024], F32, tag="phi")
                    for e in range(2):
                        jt = 2 * jp + e
                        nc.tensor.matmul(
                            phi[:, 512 * e:512 * (e + 1)],
                            vt[:, 128 * jt:128 * (jt + 1)],
                            mt0[:, sl], start=True, stop=False)
                        nc.tensor.matmul(
                            phi[:, 512 * e:512 * (e + 1)],
                            vt[:, 512 + 128 * jt:512 + 128 * (jt + 1)],
                            mt1[:, sl], start=False, stop=True)
                    sq = sqp.tile([128, 1024], F16, tag=f"sq{jp}")
                    nc.scalar.activation(sq[:], phi[:], AFT.Square,
                                         bias=0.0)
                    sqs.append(sq)
                # probs256: Re^2 + Im^2, k in [0,128) then [128,256)
                pls = []
                for h in range(2):
                    pl = prp.tile([128, 512], F16, tag=f"pr{h}")
                    nc.vector.tensor_add(pl[:], sqs[0][:, 512 * h:512 * (h + 1)],
                                         sqs[1][:, 512 * h:512 * (h + 1)])
                    pls.append(pl)
                return g, pls

            def block_preh(st):
                g, pls = st
                preh = pools['prehp'].tile([LATENT, 512], F32, tag="preh")
                nc.tensor.matmul(preh[:], at[:, 0:4], pls[0][:],
                                 start=True, stop=False)
                nc.tensor.matmul(preh[:], at[:, 4:8], pls[1][:],
                                 start=False, stop=True)
                h5 = sml.tile([LATENT, 512], F16, tag="h5")
                if g % 2 == 0:
                    nc.scalar.activation(h5[:], preh[:], AFT.Relu,
                                         bias=b1s, scale=1.0)
                else:
                    nc.vector.tensor_scalar(h5[:], preh[:], b1s, 0.0,
                                            ALU.add, ALU.max)
                return g, h5

            def block_tail(st, onat):
                g, h5 = st
                wout = pools['woutp'].tile([128, 4 * INPUT_DIM], F32,
                                           tag="wout")
                for c in range(4):
                    nc.tensor.matmul(
                        wout[:, 8 * c:8 * (c + 1)],
                        h5[:, 128 * c:128 * (c + 1)], w2s,
                        start=True, stop=True)
                nc.vector.scalar_tensor_tensor(
                    onat[:, 32 * g:32 * (g + 1)], wout[:], 1.0, b2s,
                    ALU.mult, ALU.add)

            # per-u cos/sin (ACT) + transposes (PE) + copies (DVE) into csT
            with tc.tile_pool(name="tps", bufs=2, space="PSUM") as tpsp:
                for u in range(4):
                    nc.scalar.activation(
                        cnat[:, 128 * u:128 * (u + 1)].rearrange(
                            "p (d n) -> p d n", d=2),
                        xdn[:, 2 * u:2 * u + 2, :], AFT.Sin, scale=0.5,
                        bias=halfpi[:])
                    nc.scalar.activation(
                        snat[:, 128 * u:128 * (u + 1)].rearrange(
                            "p (d n) -> p d n", d=2),
                        xdn[:, 2 * u:2 * u + 2, :], AFT.Sin, scale=0.5,
                        bias=0.0)
                    ctp = tpsp.tile([128, 128], F16, tag="tp")
                    nc.tensor.transpose(ctp[:], cnat[:, 128 * u:128 * (u + 1)],
                                        ids)
                    nc.vector.tensor_copy(csT[:, 128 * u:128 * (u + 1)], ctp[:])
                    stp = tpsp.tile([128, 128], F16, tag="tp")
                    nc.tensor.transpose(stp[:], snat[:, 128 * u:128 * (u + 1)],
                                        ids)
                    nc.vector.tensor_copy(csT[:, 512 + 128 * u:640 + 128 * u],
                                          stp[:])

            with (
                tc.tile_pool(name="phip", bufs=2, space="PSUM") as phip_,
                tc.tile_pool(name="prehp", bufs=1, space="PSUM") as prehp_,
                tc.tile_pool(name="woutp", bufs=1, space="PSUM") as woutp_,
                tc.tile_pool(name="mtap", bufs=1, space="PSUM") as mtap_,
            ):
                pools['phip'] = phip_
                pools['prehp'] = prehp_
                pools['woutp'] = woutp_
                pools['mtap'] = mtap_
                csf_gather()
                # prologue: fully stage phase 0; issue pairsA for 1 and 2
                s0 = stage_q(0)
                s_early = {1: stage_q(1), 2: stage_q(2)}
                m0a = stage_mul1(0, s0[0], s0[1])
                # PE warmup to ramp the clock while staging runs
                for _ in range(NWARM):
                    wrm = phip_.tile([128, 1024], F32, tag="phi")
                    nc.tensor.matmul(wrm[:, 0:512], vt[:, 0:128],
                                     vt[:, 0:512], start=True, stop=True)
                m0b = stage_mul2(0, m0a[1], m0a[2])
                mt = [None] * NPHASE
                a0 = sel_mtA(m0b[0], selA0)
                t0 = stage_mt(a0, m0b[1], "mt0")
                a1 = sel_mtA(m0b[0], selA1)
                mt[0] = (t0, stage_mt(a1, m0b[1], "mt1"))
                s_pend = s_early

                for p in range(NPHASE):
                    onat = sml.tile([128, 8 * NCH], F32, tag="onat")
                    nxt = p + 1
                    stage = nxt < NPHASE
                    if stage:
                        sq_ = s_pend[nxt]
                        mm1 = stage_mul1(nxt, sq_[0], sq_[1])
                    st0 = block_phi(0, mt[p][0], mt[p][1])
                    st1 = block_phi(1, mt[p][0], mt[p][1])
                    if stage:
                        hilo_n, mtB_n = stage_mul2(nxt, mm1[1], mm1[2])
                    st0 = block_preh(st0)
                    if stage:
                        aa0 = sel_mtA(hilo_n, selA0)
                        tt0 = stage_mt(aa0, mtB_n, "mt0")
                    block_tail(st0, onat)
                    st1 = block_preh(st1)
                    if stage:
                        aa1 = sel_mtA(hilo_n, selA1)
                        mt[nxt] = (tt0, stage_mt(aa1, mtB_n, "mt1"))
                        if nxt + 2 < NPHASE:
                            s_pend[nxt + 2] = stage_q(nxt + 2)
                    block_tail(st1, onat)
                    n0 = NCH * p
                    nc.sync.dma_start(
                        out.rearrange("(p n) d -> p n d", n=64)
                        [:, n0:n0 + NCH, :], onat[:])

    nc.compile()
    return nc


_NC_CACHE = []


def _get_nc():
    if not _NC_CACHE:
        _NC_CACHE.append(_build_nc())
    return _NC_CACHE[0]


def kernel(x, q_weights, w1, b1, w2, b2):
    global LAST_RESULTS
    x = np.ascontiguousarray(np.asarray(x, dtype=np.float32))
    consts = _host_consts(np.asarray(q_weights), np.asarray(w1),
                          np.asarray(b1), np.asarray(w2), np.asarray(b2))
    nc = _get_nc()
    in_maps = [
        {'xs': np.ascontiguousarray(x[i * BC:(i + 1) * BC]), **consts}
        for i in range(NCORES)
    ]
    res = run_bass_kernel_spmd(nc, in_maps, list(range(NCORES)))
    LAST_RESULTS = res
    return np.concatenate([res.results[i]['out'] for i in range(NCORES)],
                          axis=0).astype(np.float32)


# revision 27
# speedup vs baseline: 1.1311x; 1.1311x over previous
"""Trainium2 Bass kernel for nn_AutoencoderHybrid_65481071408310.

Math: the reference simulates an 8-qubit circuit per sample. The RX-encoding
layer produces a product state whose amplitudes factor as
    psi[k] = m[k] * (-i)^popcount(k),   m[k] = prod_i (cos(x_i/2) or sin(x_i/2))
and the StronglyEntanglingLayers form a fixed 256x256 unitary U that depends
only on q_weights.  Folding the popcount phases into U gives a REAL matmul
    phi = m @ V,  V = [Re(W) | Im(W)],  W = (U * (-i)^popcount)^T   (256 x 512)
then probs256 = Re^2 + Im^2 (paired), z_i = probs @ signs, and the MLP head.
signs@w1.T folds into A (256x4), contracted against probs256 with K=256.

Device pipeline per core (batch 8192, fp16 matmul operands), software
pipelined in 4 phases of 2048 samples with staging issued TWO phases ahead:
  front: cos/sin (ACT) -> 8 PE transposes -> csf_all (wire, sample) gather
  stage (per phase): pairsA replication DMA -> pairs mul (DVE) ->
    hiloA/hiloB DMAs -> hilo mul -> mtA0/mtA1/mtB DMAs -> mt0/mt1 muls
  compute (per 512-sample block): 8 phi matmuls (K=256 -> 512 wide, PSUM),
    ACT Square -> f16, DVE pair-add -> probs256, 2 A-matmuls -> preh,
    relu(+b1) ACT/DVE alternating, 4 w2 matmuls, +b2 on DVE copy-out.
"""
import sys
import numpy as np

sys.path.insert(0, '/opt/trn_rl_repo')

import concourse.bacc as bacc
import concourse.mybir as mybir
import concourse.tile as tile
from concourse.bass_utils import run_bass_kernel_spmd

F32 = mybir.dt.float32
F16 = mybir.dt.float16
AFT = mybir.ActivationFunctionType
ALU = mybir.AluOpType

NQ = 8
DIM = 256
REPS = 4
INPUT_DIM = 8
LATENT = 4
BATCH = 65536
NCORES = 8
BC = BATCH // NCORES          # 8192 samples per core
NPHASE = 8
NCH = 8                       # 128-sample chunks per phase
CH = NCH * 128                # 2048 samples per phase
NBLK = CH // 512              # 4 blocks of 512 samples per phase
NWARM = 24                    # PE clock-ramp warmup matmuls

LAST_RESULTS = None           # test harness introspection


# ---------------------------------------------------------------- host math
def _rot_mat(phi, theta, omega):
    c, s = np.cos(theta / 2), np.sin(theta / 2)
    return np.array([
        [np.exp(-0.5j * (phi + omega)) * c, -np.exp(0.5j * (phi - omega)) * s],
        [np.exp(-0.5j * (phi - omega)) * s, np.exp(0.5j * (phi + omega)) * c],
    ], dtype=np.complex128)


def _kron_list(ops):
    full = ops[0]
    for o in ops[1:]:
        full = np.kron(full, o)
    return full


def _build_entangler(qw):
    I2 = np.eye(2, dtype=np.complex128)
    P0 = np.array([[1, 0], [0, 0]], dtype=np.complex128)
    P1 = np.array([[0, 0], [0, 1]], dtype=np.complex128)
    X = np.array([[0, 1], [1, 0]], dtype=np.complex128)
    U = np.eye(DIM, dtype=np.complex128)
    for l in range(REPS):
        for i in range(NQ):
            ops = [I2] * NQ
            ops[i] = _rot_mat(*qw[l, i])
            U = _kron_list(ops) @ U
        r = (l % (NQ - 1)) + 1
        for i in range(NQ):
            t = (i + r) % NQ
            ops0 = [I2] * NQ
            ops0[i] = P0
            ops1 = [I2] * NQ
            ops1[i] = P1
            ops1[t] = X
            U = (_kron_list(ops0) + _kron_list(ops1)) @ U
    return U


def _host_consts(q_weights, w1, b1, w2, b2):
    U = _build_entangler(q_weights.astype(np.float64))
    pop = np.array([bin(k).count('1') for k in range(DIM)])
    W = (U * ((-1j) ** pop)[None, :]).T          # phi = m @ W
    V = np.concatenate([W.real, W.imag], axis=1)  # (256, 512)
    ks = np.arange(DIM)
    signs = 1.0 - 2.0 * ((ks[:, None] >> (NQ - 1 - np.arange(NQ))[None, :]) & 1)
    A = signs @ w1.T.astype(np.float64)           # (256, 4)
    vmat = V.reshape(2, 128, 512).transpose(1, 0, 2).reshape(128, 1024)
    amat = A.reshape(2, 128, LATENT).transpose(1, 0, 2).reshape(128, 2 * LATENT)
    # f16 blob: [vmat 0:1024 | amat 1024:1032 | w2.T rows0:4 1032:1040 |
    #            ident 1040:1168 | selA0 1168:1296 | selA1 1296:1424]
    blob16 = np.zeros((128, 1424), np.float16)
    blob16[:, 0:1024] = vmat.astype(np.float16)
    blob16[:, 1024:1032] = amat.astype(np.float16)
    blob16[0:LATENT, 1032:1040] = w2.T.astype(np.float16)
    blob16[:, 1040:1168] = np.eye(128, dtype=np.float16)
    ks = np.arange(128)
    selA0 = (ks[None, :] // 16 == np.arange(16)[:, None]).astype(np.float16)
    selA1 = (8 + ks[None, :] // 16 == np.arange(16)[:, None]).astype(np.float16)
    blob16[0:16, 1168:1296] = selA0
    blob16[0:16, 1296:1424] = selA1
    # f32 blob: [b2 tiled 0:32 | b1 col 32]
    blob32 = np.zeros((128, 33), np.float32)
    blob32[:, 0:32] = np.tile(b2.astype(np.float32), 4)[None, :]
    blob32[0:LATENT, 32] = b1.astype(np.float32)
    return {'blob16': np.ascontiguousarray(blob16),
            'blob32': np.ascontiguousarray(blob32)}


# ---------------------------------------------------------------- bass build
def _build_nc():
    nc = bacc.Bacc(None, target_bir_lowering=False)
    xs = nc.declare_dram_parameter("xs", [BC, INPUT_DIM], F32, isOutput=False)
    blob16 = nc.declare_dram_parameter("blob16", [128, 1424], F16, isOutput=False)
    blob32 = nc.declare_dram_parameter("blob32", [128, 33], F32, isOutput=False)
    out = nc.declare_dram_parameter("out", [BC, INPUT_DIM], F32, isOutput=True)

    with tile.TileContext(nc) as tc:
        with (
            tc.tile_pool(name="const", bufs=1) as cst,
            tc.tile_pool(name="front", bufs=1) as frt,
            tc.tile_pool(name="stage", bufs=3) as stg,
            tc.tile_pool(name="mtp", bufs=3) as mtp,
            tc.tile_pool(name="sqp", bufs=2) as sqp,
            tc.tile_pool(name="prp", bufs=2) as prp,
            tc.tile_pool(name="sml", bufs=2) as sml,
        ):
            # ---- input load first (critical path)
            xnat = frt.tile([128, BC // 16], F32)      # free = (n:64, d:8)
            nc.sync.dma_start(xnat[:], xs.rearrange("(p n) d -> p n d", n=64))
            # ---- constants (2 packed DMAs on otherwise-idle queues)
            c16 = cst.tile([128, 1424], F16)
            nc.scalar.dma_start(c16[:], blob16[:])
            c32 = cst.tile([128, 33], F32)
            nc.gpsimd.dma_start(c32[:], blob32[:])
            vt = c16[:, 0:1024]
            at = c16[:, 1024:1032]
            w2s = c16[0:LATENT, 1032:1040]
            ids = c16[:, 1040:1168]
            selA0 = c16[0:16, 1168:1296]
            selA1 = c16[0:16, 1296:1424]
            b2s = c32[:, 0:32]
            b1s = c32[0:LATENT, 32:33]
            halfpi = cst.tile([128, 1], F32)
            nc.vector.memset(halfpi[:], float(np.pi / 2))
            # ---- whole-core cos/sin, free = (d, n); sample = 64p + n
            # (warm primes the Sin table with no data dependencies)
            warm = cst.tile([1, 1], F16)
            nc.scalar.activation(warm[:], halfpi[0:1, :], AFT.Sin, scale=0.0,
                                 bias=0.0)
            cnat = frt.tile([128, BC // 16], F16)
            snat = frt.tile([128, BC // 16], F16)
            xdn = xnat.rearrange("p (n d) -> p d n", d=8)

            # ---- 8 transposes into csT: row 64*(w%2)+m,
            #      free col = 512*t + 128*(w//2) + p   (t: 0=cos 1=sin)
            csT = frt.tile([128, 1024], F16)
            # csf_all, free col = 128*m + p  -> sample 64*p + m
            #   rows 0:8  = even wires: row 2*q+tA          = cs_tA(wire 2q)
            #   rows 32:48 = odd, interleaved: 32+4q+2tA+tB = cs_tB(wire 2q+1)
            csf_all = frt.tile([48, BC], F16)
            pools = {}

            def csf_gather():
                engs = [nc.gpsimd, nc.sync, nc.scalar]
                i = 0
                for q in range(4):
                    for tA in range(2):
                        for tB in range(2):
                            src = csT[64:128, 512 * tB + 128 * q:
                                      512 * tB + 128 * (q + 1)]
                            r = 32 + 4 * q + 2 * tA + tB
                            dst = csf_all[r:r + 1, :]
                            engs[i % 3].dma_start(dst, src)
                            i += 1
                for q in range(4):
                    for tA in range(2):
                        src = csT[0:64, 512 * tA + 128 * q:
                                  512 * tA + 128 * (q + 1)]
                        dst = csf_all[2 * q + tA:2 * q + tA + 1, :]
                        engs[i % 3].dma_start(dst, src)
                        i += 1

            def stage_q(ph):
                """pairsA replication DMA for phase ph (issued 2 ahead)."""
                sl = slice(CH * ph, CH * (ph + 1))
                pairsA = stg.tile([48, CH], F16, tag="pairsA")
                nc.gpsimd.dma_start(
                    pairsA[32:48, :],
                    csf_all[0:8, sl].unsqueeze(1).broadcast_to([8, 2, CH]))
                return (pairsA, sl)

            def stage_mul1(ph, pairsA, sl):
                pairs = stg.tile([48, CH], F16, tag="pairs")
                nc.vector.tensor_mul(pairs[32:48, :], pairsA[32:48, :],
                                     csf_all[32:48, sl])
                hiloA = stg.tile([32, CH], F16, tag="hiloA")
                hiloB = stg.tile([32, CH], F16, tag="hiloB")
                for h in range(2):
                    nc.scalar.dma_start(
                        hiloA[16 * h:16 * (h + 1), :],
                        pairs[32 + 8 * h:32 + 8 * h + 4, :].unsqueeze(1)
                        .broadcast_to([4, 4, CH]))
                    eng = nc.gpsimd if h == 0 else nc.sync
                    for r in range(4):
                        eng.dma_start(
                            hiloB[16 * h + 4 * r:16 * h + 4 * (r + 1), :],
                            pairs[32 + 8 * h + 4:32 + 8 * h + 8, :])
                return pairs, hiloA, hiloB

            def stage_mul2(ph, hiloA, hiloB):
                hilo = stg.tile([32, CH], F16, tag="hilo")
                nc.vector.tensor_mul(hilo[:], hiloA[:], hiloB[:])
                # mtB: mlo tiled x8 via fanout
                mtB = stg.tile([128, CH], F16, tag="mtB")
                nc.sync.dma_start(mtB[0:16, :], hilo[16:32, :])
                nc.gpsimd.dma_start(mtB[16:32, :], mtB[0:16, :])
                nc.sync.dma_start(mtB[32:64, :], mtB[0:32, :])
                nc.gpsimd.dma_start(mtB[64:96, :], mtB[0:32, :])
                nc.sync.dma_start(mtB[96:128, :], mtB[0:32, :])
                return hilo, mtB

            def sel_mtA(hilo, sel):
                # PE row-replication: mtA[p, n] = hilo[sel-row(p), n]
                # (two matmuls: each output must fit one PSUM bank)
                mtA = pools['mtap'].tile([128, CH], F32, tag="mtA")
                for e in range(2):
                    nc.tensor.matmul(mtA[:, 512 * e:512 * (e + 1)], sel,
                                     hilo[0:16, 512 * e:512 * (e + 1)],
                                     start=True, stop=True)
                return mtA

            def stage_mt(mtA, mtB, tag):
                mt = mtp.tile([128, CH], F16, tag=tag)
                nc.vector.tensor_mul(mt[:], mtA[:], mtB[:])
                return mt

            def block_phi(g, mt0, mt1):
                phip = pools['phip']
                sl = slice(512 * g, 512 * (g + 1))
                sqs = []
                for jp in range(2):
                    phi = phip.tile([128, 1024], F32, tag="phi")
                    for e in range(2):
                        jt = 2 * jp + e
                        nc.tensor.matmul(
                            phi[:, 512 * e:512 * (e + 1)],
                            vt[:, 128 * jt:128 * (jt + 1)],
                            mt0[:, sl], start=True, stop=False)
                        nc.tensor.matmul(
                            phi[:, 512 * e:512 * (e + 1)],
                            vt[:, 512 + 128 * jt:512 + 128 * (jt + 1)],
                            mt1[:, sl], start=False, stop=True)
                    sq = sqp.tile([128, 1024], F16, tag=f"sq{jp}")
                    nc.scalar.activation(sq[:], phi[:], AFT.Square,
                                         bias=0.0)
                    sqs.append(sq)
                # probs256: Re^2 + Im^2, k in [0,128) then [128,256)
                pls = []
                for h in range(2):
                    pl = prp.tile([128, 512], F16, tag=f"pr{h}")
                    nc.vector.tensor_add(pl[:], sqs[0][:, 512 * h:512 * (h + 1)],
                                         sqs[1][:, 512 * h:512 * (h + 1)])
                    pls.append(pl)
                return g, pls

            def block_preh(st):
                g, pls = st
                preh = pools['prehp'].tile([LATENT, 512], F32, tag="preh")
                nc.tensor.matmul(preh[:], at[:, 0:4], pls[0][:],
                                 start=True, stop=False)
                nc.tensor.matmul(preh[:], at[:, 4:8], pls[1][:],
                                 start=False, stop=True)
                h5 = sml.tile([LATENT, 512], F16, tag="h5")
                if g % 2 == 0:
                    nc.scalar.activation(h5[:], preh[:], AFT.Relu,
                                         bias=b1s, scale=1.0)
                else:
                    nc.vector.tensor_scalar(h5[:], preh[:], b1s, 0.0,
                                            ALU.add, ALU.max)
                return g, h5

            def block_tail(st, onat):
                g, h5 = st
                wout = pools['woutp'].tile([128, 4 * INPUT_DIM], F32,
                                           tag="wout")
                for c in range(4):
                    nc.tensor.matmul(
                        wout[:, 8 * c:8 * (c + 1)],
                        h5[:, 128 * c:128 * (c + 1)], w2s,
                        start=True, stop=True)
                nc.vector.scalar_tensor_tensor(
                    onat[:, 32 * g:32 * (g + 1)], wout[:], 1.0, b2s,
                    ALU.mult, ALU.add)

            # per-u cos/sin (ACT) + transposes (PE) + copies (DVE) into csT
            with tc.tile_pool(name="tps", bufs=2, space="PSUM") as tpsp:
                for u in range(4):
                    nc.scalar.activation(
                        cnat[:, 128 * u:128 * (u + 1)].rearrange(
                            "p (d n) -> p d n", d=2),
                        xdn[:, 2 * u:2 * u + 2, :], AFT.Sin, scale=0.5,
                        bias=halfpi[:])
                    nc.scalar.activation(
                        snat[:, 128 * u:128 * (u + 1)].rearrange(
                            "p (d n) -> p d n", d=2),
                        xdn[:, 2 * u:2 * u + 2, :], AFT.Sin, scale=0.5,
                        bias=0.0)
                    ctp = tpsp.tile([128, 128], F16, tag="tp")
                    nc.tensor.transpose(ctp[:], cnat[:, 128 * u:128 * (u + 1)],
                                        ids)
                    nc.vector.tensor_copy(csT[:, 128 * u:128 * (u + 1)], ctp[:])
                    stp = tpsp.tile([128, 128], F16, tag="tp")
                    nc.tensor.transpose(stp[:], snat[:, 128 * u:128 * (u + 1)],
                                        ids)
                    nc.vector.tensor_copy(csT[:, 512 + 128 * u:640 + 128 * u],
                                          stp[:])

            with (
                tc.tile_pool(name="phip", bufs=2, space="PSUM") as phip_,
                tc.tile_pool(name="prehp", bufs=1, space="PSUM") as prehp_,
                tc.tile_pool(name="woutp", bufs=1, space="PSUM") as woutp_,
                tc.tile_pool(name="mtap", bufs=1, space="PSUM") as mtap_,
            ):
                pools['phip'] = phip_
                pools['prehp'] = prehp_
                pools['woutp'] = woutp_
                pools['mtap'] = mtap_
                csf_gather()
                # prologue: fully stage phase 0; issue pairsA for 1 and 2
                s0 = stage_q(0)
                s_early = {1: stage_q(1), 2: stage_q(2)}
                m0a = stage_mul1(0, s0[0], s0[1])
                # PE warmup to ramp the clock while staging runs
                for _ in range(NWARM):
                    wrm = phip_.tile([128, 1024], F32, tag="phi")
                    nc.tensor.matmul(wrm[:, 0:512], vt[:, 0:128],
                                     vt[:, 0:512], start=True, stop=True)
                m0b = stage_mul2(0, m0a[1], m0a[2])
                mt = [None] * NPHASE
                a0 = sel_mtA(m0b[0], selA0)
                t0 = stage_mt(a0, m0b[1], "mt0")
                a1 = sel_mtA(m0b[0], selA1)
                mt[0] = (t0, stage_mt(a1, m0b[1], "mt1"))
                s_pend = s_early

                for p in range(NPHASE):
                    onat = sml.tile([128, 8 * NCH], F32, tag="onat")
                    nxt = p + 1
                    stage = nxt < NPHASE
                    if stage:
                        sq_ = s_pend[nxt]
                        mm1 = stage_mul1(nxt, sq_[0], sq_[1])
                    st0 = block_phi(0, mt[p][0], mt[p][1])
                    st1 = block_phi(1, mt[p][0], mt[p][1])
                    if stage:
                        hilo_n, mtB_n = stage_mul2(nxt, mm1[1], mm1[2])
                    st0 = block_preh(st0)
                    if stage:
                        aa0 = sel_mtA(hilo_n, selA0)
                        tt0 = stage_mt(aa0, mtB_n, "mt0")
                    block_tail(st0, onat)
                    st1 = block_preh(st1)
                    if stage:
                        aa1 = sel_mtA(hilo_n, selA1)
                        mt[nxt] = (tt0, stage_mt(aa1, mtB_n, "mt1"))
                        if nxt + 2 < NPHASE:
                            s_pend[nxt + 2] = stage_q(nxt + 2)
                    block_tail(st1, onat)
                    n0 = NCH * p
                    nc.sync.dma_start(
                        out.rearrange("(p n) d -> p n d", n=64)
                        [:, n0:n0 + NCH, :], onat[:])

    nc.compile()
    return nc


_NC_CACHE = []


def _get_nc():
    if not _NC_CACHE:
        _NC_CACHE.append(_build_nc())
    return _NC_CACHE[0]


def kernel(x, q_weights, w1, b1, w2, b2):
    global LAST_RESULTS
    x = np.ascontiguousarray(np.asarray(x, dtype=np.float32))
    consts = _host_consts(np.asarray(q_weights), np.asarray(w1),
                          np.asarray(b1), np.asarray(w2), np.asarray(b2))
    nc = _get_nc()
    in_maps = [
        {'xs': np.ascontiguousarray(x[i * BC:(i + 1) * BC]), **consts}
        for i in range(NCORES)
    ]
    res = run_bass_kernel_spmd(nc, in_maps, list(range(NCORES)))
    LAST_RESULTS = res
    return np.concatenate([res.results[i]['out'] for i in range(NCORES)],
                          axis=0).astype(np.float32)


# revision 28
# speedup vs baseline: 1.1584x; 1.0241x over previous
"""Trainium2 Bass kernel for nn_AutoencoderHybrid_65481071408310.

Math: the reference simulates an 8-qubit circuit per sample. The RX-encoding
layer produces a product state whose amplitudes factor as
    psi[k] = m[k] * (-i)^popcount(k),   m[k] = prod_i (cos(x_i/2) or sin(x_i/2))
and the StronglyEntanglingLayers form a fixed 256x256 unitary U that depends
only on q_weights.  Folding the popcount phases into U gives a REAL matmul
    phi = m @ V,  V = [Re(W) | Im(W)],  W = (U * (-i)^popcount)^T   (256 x 512)
then probs256 = Re^2 + Im^2 (paired), z_i = probs @ signs, and the MLP head.
signs@w1.T folds into A (256x4), contracted against probs256 with K=256.

Device pipeline per core (batch 8192, fp16 matmul operands), software
pipelined in 4 phases of 2048 samples with staging issued TWO phases ahead:
  front: cos/sin (ACT) -> 8 PE transposes -> csf_all (wire, sample) gather
  stage (per phase): pairsA replication DMA -> pairs mul (DVE) ->
    hiloA/hiloB DMAs -> hilo mul -> mtA0/mtA1/mtB DMAs -> mt0/mt1 muls
  compute (per 512-sample block): 8 phi matmuls (K=256 -> 512 wide, PSUM),
    ACT Square -> f16, DVE pair-add -> probs256, 2 A-matmuls -> preh,
    relu(+b1) ACT/DVE alternating, 4 w2 matmuls, +b2 on DVE copy-out.
"""
import sys
import numpy as np

sys.path.insert(0, '/opt/trn_rl_repo')

import concourse.bacc as bacc
import concourse.mybir as mybir
import concourse.tile as tile
from concourse.bass_utils import run_bass_kernel_spmd

F32 = mybir.dt.float32
F16 = mybir.dt.float16
AFT = mybir.ActivationFunctionType
ALU = mybir.AluOpType

NQ = 8
DIM = 256
REPS = 4
INPUT_DIM = 8
LATENT = 4
BATCH = 65536
NCORES = 8
BC = BATCH // NCORES          # 8192 samples per core
NPHASE = 8
NCH = 8                       # 128-sample chunks per phase
CH = NCH * 128                # 2048 samples per phase
NBLK = CH // 512              # 4 blocks of 512 samples per phase
NWARM = 40                    # PE clock-ramp warmup matmuls

LAST_RESULTS = None           # test harness introspection


# ---------------------------------------------------------------- host math
def _rot_mat(phi, theta, omega):
    c, s = np.cos(theta / 2), np.sin(theta / 2)
    return np.array([
        [np.exp(-0.5j * (phi + omega)) * c, -np.exp(0.5j * (phi - omega)) * s],
        [np.exp(-0.5j * (phi - omega)) * s, np.exp(0.5j * (phi + omega)) * c],
    ], dtype=np.complex128)


def _kron_list(ops):
    full = ops[0]
    for o in ops[1:]:
        full = np.kron(full, o)
    return full


def _build_entangler(qw):
    I2 = np.eye(2, dtype=np.complex128)
    P0 = np.array([[1, 0], [0, 0]], dtype=np.complex128)
    P1 = np.array([[0, 0], [0, 1]], dtype=np.complex128)
    X = np.array([[0, 1], [1, 0]], dtype=np.complex128)
    U = np.eye(DIM, dtype=np.complex128)
    for l in range(REPS):
        for i in range(NQ):
            ops = [I2] * NQ
            ops[i] = _rot_mat(*qw[l, i])
            U = _kron_list(ops) @ U
        r = (l % (NQ - 1)) + 1
        for i in range(NQ):
            t = (i + r) % NQ
            ops0 = [I2] * NQ
            ops0[i] = P0
            ops1 = [I2] * NQ
            ops1[i] = P1
            ops1[t] = X
            U = (_kron_list(ops0) + _kron_list(ops1)) @ U
    return U


def _host_consts(q_weights, w1, b1, w2, b2):
    U = _build_entangler(q_weights.astype(np.float64))
    pop = np.array([bin(k).count('1') for k in range(DIM)])
    W = (U * ((-1j) ** pop)[None, :]).T          # phi = m @ W
    V = np.concatenate([W.real, W.imag], axis=1)  # (256, 512)
    ks = np.arange(DIM)
    signs = 1.0 - 2.0 * ((ks[:, None] >> (NQ - 1 - np.arange(NQ))[None, :]) & 1)
    A = signs @ w1.T.astype(np.float64)           # (256, 4)
    vmat = V.reshape(2, 128, 512).transpose(1, 0, 2).reshape(128, 1024)
    amat = A.reshape(2, 128, LATENT).transpose(1, 0, 2).reshape(128, 2 * LATENT)
    # f16 blob: [vmat 0:1024 | amat 1024:1032 | w2.T rows0:4 1032:1040 |
    #            ident 1040:1168 | selA0 1168:1296 | selA1 1296:1424]
    blob16 = np.zeros((128, 1424), np.float16)
    blob16[:, 0:1024] = vmat.astype(np.float16)
    blob16[:, 1024:1032] = amat.astype(np.float16)
    blob16[0:LATENT, 1032:1040] = w2.T.astype(np.float16)
    blob16[:, 1040:1168] = np.eye(128, dtype=np.float16)
    ks = np.arange(128)
    selA0 = (ks[None, :] // 16 == np.arange(16)[:, None]).astype(np.float16)
    selA1 = (8 + ks[None, :] // 16 == np.arange(16)[:, None]).astype(np.float16)
    blob16[0:16, 1168:1296] = selA0
    blob16[0:16, 1296:1424] = selA1
    # f32 blob: [b2 tiled 0:32 | b1 col 32]
    blob32 = np.zeros((128, 33), np.float32)
    blob32[:, 0:32] = np.tile(b2.astype(np.float32), 4)[None, :]
    blob32[0:LATENT, 32] = b1.astype(np.float32)
    return {'blob16': np.ascontiguousarray(blob16),
            'blob32': np.ascontiguousarray(blob32)}


# ---------------------------------------------------------------- bass build
def _build_nc():
    nc = bacc.Bacc(None, target_bir_lowering=False)
    xs = nc.declare_dram_parameter("xs", [BC, INPUT_DIM], F32, isOutput=False)
    blob16 = nc.declare_dram_parameter("blob16", [128, 1424], F16, isOutput=False)
    blob32 = nc.declare_dram_parameter("blob32", [128, 33], F32, isOutput=False)
    out = nc.declare_dram_parameter("out", [BC, INPUT_DIM], F32, isOutput=True)

    with tile.TileContext(nc) as tc:
        with (
            tc.tile_pool(name="const", bufs=1) as cst,
            tc.tile_pool(name="front", bufs=1) as frt,
            tc.tile_pool(name="stage", bufs=3) as stg,
            tc.tile_pool(name="mtp", bufs=3) as mtp,
            tc.tile_pool(name="sqp", bufs=2) as sqp,
            tc.tile_pool(name="prp", bufs=2) as prp,
            tc.tile_pool(name="sml", bufs=2) as sml,
        ):
            # ---- input load first (critical path)
            xnat = frt.tile([128, BC // 16], F32)      # free = (n:64, d:8)
            nc.sync.dma_start(xnat[:], xs.rearrange("(p n) d -> p n d", n=64))
            # ---- constants (2 packed DMAs on otherwise-idle queues)
            c16 = cst.tile([128, 1424], F16)
            nc.scalar.dma_start(c16[:], blob16[:])
            c32 = cst.tile([128, 33], F32)
            nc.gpsimd.dma_start(c32[:], blob32[:])
            vt = c16[:, 0:1024]
            at = c16[:, 1024:1032]
            w2s = c16[0:LATENT, 1032:1040]
            ids = c16[:, 1040:1168]
            selA0 = c16[0:16, 1168:1296]
            selA1 = c16[0:16, 1296:1424]
            b2s = c32[:, 0:32]
            b1s = c32[0:LATENT, 32:33]
            halfpi = cst.tile([128, 1], F32)
            nc.vector.memset(halfpi[:], float(np.pi / 2))
            # ---- whole-core cos/sin, free = (d, n); sample = 64p + n
            # (warm primes the Sin table with no data dependencies)
            warm = cst.tile([1, 1], F16)
            nc.scalar.activation(warm[:], halfpi[0:1, :], AFT.Sin, scale=0.0,
                                 bias=0.0)
            cnat = frt.tile([128, BC // 16], F16)
            snat = frt.tile([128, BC // 16], F16)
            xdn = xnat.rearrange("p (n d) -> p d n", d=8)

            # ---- 8 transposes into csT: row 64*(w%2)+m,
            #      free col = 512*t + 128*(w//2) + p   (t: 0=cos 1=sin)
            csT = frt.tile([128, 1024], F16)
            # csf_all, free col = 128*m + p  -> sample 64*p + m
            #   rows 0:8  = even wires: row 2*q+tA          = cs_tA(wire 2q)
            #   rows 32:48 = odd, interleaved: 32+4q+2tA+tB = cs_tB(wire 2q+1)
            csf_all = frt.tile([48, BC], F16)
            pools = {}

            def csf_gather():
                engs = [nc.gpsimd, nc.sync, nc.scalar]
                i = 0
                for q in range(4):
                    for tA in range(2):
                        for tB in range(2):
                            src = csT[64:128, 512 * tB + 128 * q:
                                      512 * tB + 128 * (q + 1)]
                            r = 32 + 4 * q + 2 * tA + tB
                            dst = csf_all[r:r + 1, :]
                            engs[i % 3].dma_start(dst, src)
                            i += 1
                for q in range(4):
                    for tA in range(2):
                        src = csT[0:64, 512 * tA + 128 * q:
                                  512 * tA + 128 * (q + 1)]
                        dst = csf_all[2 * q + tA:2 * q + tA + 1, :]
                        engs[i % 3].dma_start(dst, src)
                        i += 1

            def stage_q(ph):
                """pairsA replication DMA for phase ph (issued 2 ahead)."""
                sl = slice(CH * ph, CH * (ph + 1))
                pairsA = stg.tile([48, CH], F16, tag="pairsA")
                nc.gpsimd.dma_start(
                    pairsA[32:48, :],
                    csf_all[0:8, sl].unsqueeze(1).broadcast_to([8, 2, CH]))
                return (pairsA, sl)

            def stage_mul1(ph, pairsA, sl):
                pairs = stg.tile([48, CH], F16, tag="pairs")
                nc.vector.tensor_mul(pairs[32:48, :], pairsA[32:48, :],
                                     csf_all[32:48, sl])
                hiloA = stg.tile([32, CH], F16, tag="hiloA")
                hiloB = stg.tile([32, CH], F16, tag="hiloB")
                for h in range(2):
                    nc.scalar.dma_start(
                        hiloA[16 * h:16 * (h + 1), :],
                        pairs[32 + 8 * h:32 + 8 * h + 4, :].unsqueeze(1)
                        .broadcast_to([4, 4, CH]))
                    eng = nc.gpsimd if h == 0 else nc.sync
                    for r in range(4):
                        eng.dma_start(
                            hiloB[16 * h + 4 * r:16 * h + 4 * (r + 1), :],
                            pairs[32 + 8 * h + 4:32 + 8 * h + 8, :])
                return pairs, hiloA, hiloB

            def stage_mul2(ph, hiloA, hiloB):
                hilo = stg.tile([32, CH], F16, tag="hilo")
                nc.vector.tensor_mul(hilo[:], hiloA[:], hiloB[:])
                # mtB: mlo tiled x8 via fanout
                mtB = stg.tile([128, CH], F16, tag="mtB")
                nc.sync.dma_start(mtB[0:16, :], hilo[16:32, :])
                nc.gpsimd.dma_start(mtB[16:32, :], mtB[0:16, :])
                nc.sync.dma_start(mtB[32:64, :], mtB[0:32, :])
                nc.gpsimd.dma_start(mtB[64:96, :], mtB[0:32, :])
                nc.sync.dma_start(mtB[96:128, :], mtB[0:32, :])
                return hilo, mtB

            def sel_mtA(hilo, sel):
                # PE row-replication: mtA[p, n] = hilo[sel-row(p), n]
                # (two matmuls: each output must fit one PSUM bank)
                mtA = pools['mtap'].tile([128, CH], F32, tag="mtA")
                for e in range(2):
                    nc.tensor.matmul(mtA[:, 512 * e:512 * (e + 1)], sel,
                                     hilo[0:16, 512 * e:512 * (e + 1)],
                                     start=True, stop=True)
                return mtA

            def stage_mt(mtA, mtB, tag):
                mt = mtp.tile([128, CH], F16, tag=tag)
                nc.vector.tensor_mul(mt[:], mtA[:], mtB[:])
                return mt

            def block_phi(g, mt0, mt1):
                phip = pools['phip']
                sl = slice(512 * g, 512 * (g + 1))
                sqs = []
                for jp in range(2):
                    phi = phip.tile([128, 1024], F32, tag="phi")
                    for e in range(2):
                        jt = 2 * jp + e
                        nc.tensor.matmul(
                            phi[:, 512 * e:512 * (e + 1)],
                            vt[:, 128 * jt:128 * (jt + 1)],
                            mt0[:, sl], start=True, stop=False)
                        nc.tensor.matmul(
                            phi[:, 512 * e:512 * (e + 1)],
                            vt[:, 512 + 128 * jt:512 + 128 * (jt + 1)],
                            mt1[:, sl], start=False, stop=True)
                    sq = sqp.tile([128, 1024], F16, tag=f"sq{jp}")
                    nc.scalar.activation(sq[:], phi[:], AFT.Square,
                                         bias=0.0)
                    sqs.append(sq)
                # probs256: Re^2 + Im^2, k in [0,128) then [128,256)
                pls = []
                for h in range(2):
                    pl = prp.tile([128, 512], F16, tag=f"pr{h}")
                    nc.vector.tensor_add(pl[:], sqs[0][:, 512 * h:512 * (h + 1)],
                                         sqs[1][:, 512 * h:512 * (h + 1)])
                    pls.append(pl)
                return g, pls

            def block_preh(st):
                g, pls = st
                preh = pools['prehp'].tile([LATENT, 512], F32, tag="preh")
                nc.tensor.matmul(preh[:], at[:, 0:4], pls[0][:],
                                 start=True, stop=False)
                nc.tensor.matmul(preh[:], at[:, 4:8], pls[1][:],
                                 start=False, stop=True)
                h5 = sml.tile([LATENT, 512], F16, tag="h5")
                if g % 2 == 0:
                    nc.scalar.activation(h5[:], preh[:], AFT.Relu,
                                         bias=b1s, scale=1.0)
                else:
                    nc.vector.tensor_scalar(h5[:], preh[:], b1s, 0.0,
                                            ALU.add, ALU.max)
                return g, h5

            def block_tail(st, onat):
                g, h5 = st
                wout = pools['woutp'].tile([128, 4 * INPUT_DIM], F32,
                                           tag="wout")
                for c in range(4):
                    nc.tensor.matmul(
                        wout[:, 8 * c:8 * (c + 1)],
                        h5[:, 128 * c:128 * (c + 1)], w2s,
                        start=True, stop=True)
                nc.vector.scalar_tensor_tensor(
                    onat[:, 32 * g:32 * (g + 1)], wout[:], 1.0, b2s,
                    ALU.mult, ALU.add)

            # per-u cos/sin (ACT) + transposes (PE) + copies (DVE) into csT
            with tc.tile_pool(name="tps", bufs=2, space="PSUM") as tpsp:
                for u in range(4):
                    nc.scalar.activation(
                        cnat[:, 128 * u:128 * (u + 1)].rearrange(
                            "p (d n) -> p d n", d=2),
                        xdn[:, 2 * u:2 * u + 2, :], AFT.Sin, scale=0.5,
                        bias=halfpi[:])
                    nc.scalar.activation(
                        snat[:, 128 * u:128 * (u + 1)].rearrange(
                            "p (d n) -> p d n", d=2),
                        xdn[:, 2 * u:2 * u + 2, :], AFT.Sin, scale=0.5,
                        bias=0.0)
                    ctp = tpsp.tile([128, 128], F16, tag="tp")
                    nc.tensor.transpose(ctp[:], cnat[:, 128 * u:128 * (u + 1)],
                                        ids)
                    nc.vector.tensor_copy(csT[:, 128 * u:128 * (u + 1)], ctp[:])
                    stp = tpsp.tile([128, 128], F16, tag="tp")
                    nc.tensor.transpose(stp[:], snat[:, 128 * u:128 * (u + 1)],
                                        ids)
                    nc.vector.tensor_copy(csT[:, 512 + 128 * u:640 + 128 * u],
                                          stp[:])

            with (
                tc.tile_pool(name="phip", bufs=2, space="PSUM") as phip_,
                tc.tile_pool(name="prehp", bufs=1, space="PSUM") as prehp_,
                tc.tile_pool(name="woutp", bufs=1, space="PSUM") as woutp_,
                tc.tile_pool(name="mtap", bufs=1, space="PSUM") as mtap_,
            ):
                pools['phip'] = phip_
                pools['prehp'] = prehp_
                pools['woutp'] = woutp_
                pools['mtap'] = mtap_
                csf_gather()
                # prologue: fully stage phase 0; issue pairsA for 1 and 2
                s0 = stage_q(0)
                s_early = {1: stage_q(1), 2: stage_q(2)}
                m0a = stage_mul1(0, s0[0], s0[1])
                # PE warmup to ramp the clock while staging runs
                for _ in range(NWARM):
                    wrm = phip_.tile([128, 1024], F32, tag="phi")
                    nc.tensor.matmul(wrm[:, 0:512], vt[:, 0:128],
                                     vt[:, 0:512], start=True, stop=True)
                m0b = stage_mul2(0, m0a[1], m0a[2])
                mt = [None] * NPHASE
                a0 = sel_mtA(m0b[0], selA0)
                t0 = stage_mt(a0, m0b[1], "mt0")
                a1 = sel_mtA(m0b[0], selA1)
                mt[0] = (t0, stage_mt(a1, m0b[1], "mt1"))
                s_pend = s_early

                for p in range(NPHASE):
                    onat = sml.tile([128, 8 * NCH], F32, tag="onat")
                    nxt = p + 1
                    stage = nxt < NPHASE
                    if stage:
                        sq_ = s_pend[nxt]
                        mm1 = stage_mul1(nxt, sq_[0], sq_[1])
                    st0 = block_phi(0, mt[p][0], mt[p][1])
                    st1 = block_phi(1, mt[p][0], mt[p][1])
                    if stage:
                        hilo_n, mtB_n = stage_mul2(nxt, mm1[1], mm1[2])
                    st0 = block_preh(st0)
                    if stage:
                        aa0 = sel_mtA(hilo_n, selA0)
                        tt0 = stage_mt(aa0, mtB_n, "mt0")
                    block_tail(st0, onat)
                    st1 = block_preh(st1)
                    if stage:
                        aa1 = sel_mtA(hilo_n, selA1)
                        mt[nxt] = (tt0, stage_mt(aa1, mtB_n, "mt1"))
                        if nxt + 2 < NPHASE:
                            s_pend[nxt + 2] = stage_q(nxt + 2)
                    block_tail(st1, onat)
                    n0 = NCH * p
                    nc.sync.dma_start(
                        out.rearrange("(p n) d -> p n d", n=64)
                        [:, n0:n0 + NCH, :], onat[:])

    nc.compile()
    return nc


_NC_CACHE = []


def _get_nc():
    if not _NC_CACHE:
        _NC_CACHE.append(_build_nc())
    return _NC_CACHE[0]


def kernel(x, q_weights, w1, b1, w2, b2):
    global LAST_RESULTS
    x = np.ascontiguousarray(np.asarray(x, dtype=np.float32))
    consts = _host_consts(np.asarray(q_weights), np.asarray(w1),
                          np.asarray(b1), np.asarray(w2), np.asarray(b2))
    nc = _get_nc()
    in_maps = [
        {'xs': np.ascontiguousarray(x[i * BC:(i + 1) * BC]), **consts}
        for i in range(NCORES)
    ]
    res = run_bass_kernel_spmd(nc, in_maps, list(range(NCORES)))
    LAST_RESULTS = res
    return np.concatenate([res.results[i]['out'] for i in range(NCORES)],
                          axis=0).astype(np.float32)


# revision 30
# speedup vs baseline: 1.1638x; 1.0047x over previous
"""Trainium2 Bass kernel for nn_AutoencoderHybrid_65481071408310.

Math: the reference simulates an 8-qubit circuit per sample. The RX-encoding
layer produces a product state whose amplitudes factor as
    psi[k] = m[k] * (-i)^popcount(k),   m[k] = prod_i (cos(x_i/2) or sin(x_i/2))
and the StronglyEntanglingLayers form a fixed 256x256 unitary U that depends
only on q_weights.  Folding the popcount phases into U gives a REAL matmul
    phi = m @ V,  V = [Re(W) | Im(W)],  W = (U * (-i)^popcount)^T   (256 x 512)
then probs256 = Re^2 + Im^2 (paired), z_i = probs @ signs, and the MLP head.
signs@w1.T folds into A (256x4), contracted against probs256 with K=256.

Device pipeline per core (batch 8192, fp16 matmul operands), software
pipelined in 4 phases of 2048 samples with staging issued TWO phases ahead:
  front: cos/sin (ACT) -> 8 PE transposes -> csf_all (wire, sample) gather
  stage (per phase): pairsA replication DMA -> pairs mul (DVE) ->
    hiloA/hiloB DMAs -> hilo mul -> mtA0/mtA1/mtB DMAs -> mt0/mt1 muls
  compute (per 512-sample block): 8 phi matmuls (K=256 -> 512 wide, PSUM),
    ACT Square -> f16, DVE pair-add -> probs256, 2 A-matmuls -> preh,
    relu(+b1) ACT/DVE alternating, 4 w2 matmuls, +b2 on DVE copy-out.
"""
import sys
import numpy as np

sys.path.insert(0, '/opt/trn_rl_repo')

import concourse.bacc as bacc
import concourse.mybir as mybir
import concourse.tile as tile
from concourse.bass_utils import run_bass_kernel_spmd

F32 = mybir.dt.float32
F16 = mybir.dt.float16
AFT = mybir.ActivationFunctionType
ALU = mybir.AluOpType

NQ = 8
DIM = 256
REPS = 4
INPUT_DIM = 8
LATENT = 4
BATCH = 65536
NCORES = 8
BC = BATCH // NCORES          # 8192 samples per core
NPHASE = 8
NCH = 8                       # 128-sample chunks per phase
CH = NCH * 128                # 2048 samples per phase
NBLK = CH // 512              # 4 blocks of 512 samples per phase
NWARM = 28                    # PE clock-ramp warmup matmuls

LAST_RESULTS = None           # test harness introspection


# ---------------------------------------------------------------- host math
def _rot_mat(phi, theta, omega):
    c, s = np.cos(theta / 2), np.sin(theta / 2)
    return np.array([
        [np.exp(-0.5j * (phi + omega)) * c, -np.exp(0.5j * (phi - omega)) * s],
        [np.exp(-0.5j * (phi - omega)) * s, np.exp(0.5j * (phi + omega)) * c],
    ], dtype=np.complex128)


def _kron_list(ops):
    full = ops[0]
    for o in ops[1:]:
        full = np.kron(full, o)
    return full


def _build_entangler(qw):
    I2 = np.eye(2, dtype=np.complex128)
    P0 = np.array([[1, 0], [0, 0]], dtype=np.complex128)
    P1 = np.array([[0, 0], [0, 1]], dtype=np.complex128)
    X = np.array([[0, 1], [1, 0]], dtype=np.complex128)
    U = np.eye(DIM, dtype=np.complex128)
    for l in range(REPS):
        for i in range(NQ):
            ops = [I2] * NQ
            ops[i] = _rot_mat(*qw[l, i])
            U = _kron_list(ops) @ U
        r = (l % (NQ - 1)) + 1
        for i in range(NQ):
            t = (i + r) % NQ
            ops0 = [I2] * NQ
            ops0[i] = P0
            ops1 = [I2] * NQ
            ops1[i] = P1
            ops1[t] = X
            U = (_kron_list(ops0) + _kron_list(ops1)) @ U
    return U


def _host_consts(q_weights, w1, b1, w2, b2):
    U = _build_entangler(q_weights.astype(np.float64))
    pop = np.array([bin(k).count('1') for k in range(DIM)])
    W = (U * ((-1j) ** pop)[None, :]).T          # phi = m @ W
    V = np.concatenate([W.real, W.imag], axis=1)  # (256, 512)
    ks = np.arange(DIM)
    signs = 1.0 - 2.0 * ((ks[:, None] >> (NQ - 1 - np.arange(NQ))[None, :]) & 1)
    A = signs @ w1.T.astype(np.float64)           # (256, 4)
    vmat = V.reshape(2, 128, 512).transpose(1, 0, 2).reshape(128, 1024)
    amat = A.reshape(2, 128, LATENT).transpose(1, 0, 2).reshape(128, 2 * LATENT)
    # f16 blob: [vmat 0:1024 | amat 1024:1032 | w2.T rows0:4 1032:1040 |
    #            ident 1040:1168 | selA0 1168:1296 | selA1 1296:1424]
    blob16 = np.zeros((128, 1424), np.float16)
    blob16[:, 0:1024] = vmat.astype(np.float16)
    blob16[:, 1024:1032] = amat.astype(np.float16)
    blob16[0:LATENT, 1032:1040] = w2.T.astype(np.float16)
    blob16[:, 1040:1168] = np.eye(128, dtype=np.float16)
    ks = np.arange(128)
    selA0 = (ks[None, :] // 16 == np.arange(16)[:, None]).astype(np.float16)
    selA1 = (8 + ks[None, :] // 16 == np.arange(16)[:, None]).astype(np.float16)
    blob16[0:16, 1168:1296] = selA0
    blob16[0:16, 1296:1424] = selA1
    # f32 blob: [b2 tiled 0:32 | b1 col 32]
    blob32 = np.zeros((128, 33), np.float32)
    blob32[:, 0:32] = np.tile(b2.astype(np.float32), 4)[None, :]
    blob32[0:LATENT, 32] = b1.astype(np.float32)
    return {'blob16': np.ascontiguousarray(blob16),
            'blob32': np.ascontiguousarray(blob32)}


# ---------------------------------------------------------------- bass build
def _build_nc():
    nc = bacc.Bacc(None, target_bir_lowering=False)
    xs = nc.declare_dram_parameter("xs", [BC, INPUT_DIM], F32, isOutput=False)
    blob16 = nc.declare_dram_parameter("blob16", [128, 1424], F16, isOutput=False)
    blob32 = nc.declare_dram_parameter("blob32", [128, 33], F32, isOutput=False)
    out = nc.declare_dram_parameter("out", [BC, INPUT_DIM], F32, isOutput=True)

    with tile.TileContext(nc) as tc:
        with (
            tc.tile_pool(name="const", bufs=1) as cst,
            tc.tile_pool(name="front", bufs=1) as frt,
            tc.tile_pool(name="stage", bufs=3) as stg,
            tc.tile_pool(name="mtp", bufs=3) as mtp,
            tc.tile_pool(name="sqp", bufs=2) as sqp,
            tc.tile_pool(name="prp", bufs=2) as prp,
            tc.tile_pool(name="sml", bufs=2) as sml,
        ):
            # ---- input load first (critical path)
            xnat = frt.tile([128, BC // 16], F32)      # free = (n:64, d:8)
            nc.sync.dma_start(xnat[:], xs.rearrange("(p n) d -> p n d", n=64))
            # ---- constants (2 packed DMAs on otherwise-idle queues)
            c16 = cst.tile([128, 1424], F16)
            nc.scalar.dma_start(c16[:], blob16[:])
            c32 = cst.tile([128, 33], F32)
            nc.gpsimd.dma_start(c32[:], blob32[:])
            vt = c16[:, 0:1024]
            at = c16[:, 1024:1032]
            w2s = c16[0:LATENT, 1032:1040]
            ids = c16[:, 1040:1168]
            selA0 = c16[0:16, 1168:1296]
            selA1 = c16[0:16, 1296:1424]
            b2s = c32[:, 0:32]
            b1s = c32[0:LATENT, 32:33]
            halfpi = cst.tile([128, 1], F32)
            nc.vector.memset(halfpi[:], float(np.pi / 2))
            # ---- whole-core cos/sin, free = (d, n); sample = 64p + n
            # (warm primes the Sin table with no data dependencies)
            warm = cst.tile([1, 1], F16)
            nc.scalar.activation(warm[:], halfpi[0:1, :], AFT.Sin, scale=0.0,
                                 bias=0.0)
            cnat = frt.tile([128, BC // 16], F16)
            snat = frt.tile([128, BC // 16], F16)
            xdn = xnat.rearrange("p (n d) -> p d n", d=8)

            # ---- 8 transposes into csT: row 64*(w%2)+m,
            #      free col = 512*t + 128*(w//2) + p   (t: 0=cos 1=sin)
            csT = frt.tile([128, 1024], F16)
            # csf_all, free col = 128*m + p  -> sample 64*p + m
            #   rows 0:8  = even wires: row 2*q+tA          = cs_tA(wire 2q)
            #   rows 32:48 = odd, interleaved: 32+4q+2tA+tB = cs_tB(wire 2q+1)
            csf_all = frt.tile([72, BC], F16)
            pools = {}

            def csf_gather():
                engs = [nc.gpsimd, nc.sync, nc.scalar]
                i = 0
                for qg in range(2):            # q-pair group: {0,1} then {2,3}
                    for q in (2 * qg, 2 * qg + 1):
                        for tA in range(2):
                            src = csT[0:64, 512 * tA + 128 * q:
                                      512 * tA + 128 * (q + 1)]
                            dst = csf_all[2 * q + tA:2 * q + tA + 1, :]
                            engs[i % 3].dma_start(dst, src)
                            i += 1
                    for q in (2 * qg, 2 * qg + 1):
                        for tA in range(2):
                            for tB in range(2):
                                src = csT[64:128, 512 * tB + 128 * q:
                                          512 * tB + 128 * (q + 1)]
                                r = 32 * (qg + 1) + 4 * (q - 2 * qg) + \
                                    2 * tA + tB
                                dst = csf_all[r:r + 1, :]
                                engs[i % 3].dma_start(dst, src)
                                i += 1

            def stage_q(ph):
                """pairsA replication DMAs for phase ph (issued 2 ahead)."""
                sl = slice(CH * ph, CH * (ph + 1))
                pairsA = stg.tile([72, CH], F16, tag="pairsA")
                nc.gpsimd.dma_start(
                    pairsA[32:40, :],
                    csf_all[0:4, sl].unsqueeze(1).broadcast_to([4, 2, CH]))
                nc.gpsimd.dma_start(
                    pairsA[64:72, :],
                    csf_all[4:8, sl].unsqueeze(1).broadcast_to([4, 2, CH]))
                return (pairsA, sl)

            def stage_mul1(ph, pairsA, sl):
                pairs = stg.tile([72, CH], F16, tag="pairs")
                hiloA = stg.tile([32, CH], F16, tag="hiloA")
                hiloB = stg.tile([32, CH], F16, tag="hiloB")
                for h in range(2):
                    b = 32 * (h + 1)
                    nc.vector.tensor_mul(pairs[b:b + 8, :],
                                         pairsA[b:b + 8, :],
                                         csf_all[b:b + 8, sl])
                    nc.scalar.dma_start(
                        hiloA[16 * h:16 * (h + 1), :],
                        pairs[b:b + 4, :].unsqueeze(1)
                        .broadcast_to([4, 4, CH]))
                    eng = nc.gpsimd if h == 0 else nc.sync
                    for r in range(4):
                        eng.dma_start(
                            hiloB[16 * h + 4 * r:16 * h + 4 * (r + 1), :],
                            pairs[b + 4:b + 8, :])
                return pairs, hiloA, hiloB

            def stage_mul2(ph, hiloA, hiloB):
                hilo = stg.tile([32, CH], F16, tag="hilo")
                nc.vector.tensor_mul(hilo[:], hiloA[:], hiloB[:])
                # mtB: mlo tiled x8 via fanout
                mtB = stg.tile([128, CH], F16, tag="mtB")
                nc.sync.dma_start(mtB[0:16, :], hilo[16:32, :])
                nc.gpsimd.dma_start(mtB[16:32, :], mtB[0:16, :])
                nc.sync.dma_start(mtB[32:64, :], mtB[0:32, :])
                nc.gpsimd.dma_start(mtB[64:96, :], mtB[0:32, :])
                nc.sync.dma_start(mtB[96:128, :], mtB[0:32, :])
                return hilo, mtB

            def sel_mtA(hilo, sel):
                # PE row-replication: mtA[p, n] = hilo[sel-row(p), n]
                # (two matmuls: each output must fit one PSUM bank)
                mtA = pools['mtap'].tile([128, CH], F32, tag="mtA")
                for e in range(2):
                    nc.tensor.matmul(mtA[:, 512 * e:512 * (e + 1)], sel,
                                     hilo[0:16, 512 * e:512 * (e + 1)],
                                     start=True, stop=True)
                return mtA

            def stage_mt(mtA, mtB, tag):
                mt = mtp.tile([128, CH], F16, tag=tag)
                nc.vector.tensor_mul(mt[:], mtA[:], mtB[:])
                return mt

            def block_phi(g, mt0, mt1):
                phip = pools['phip']
                sl = slice(512 * g, 512 * (g + 1))
                sqs = []
                for jp in range(2):
                    phi = phip.tile([128, 1024], F32, tag="phi")
                    for e in range(2):
                        jt = 2 * jp + e
                        nc.tensor.matmul(
                            phi[:, 512 * e:512 * (e + 1)],
                            vt[:, 128 * jt:128 * (jt + 1)],
                            mt0[:, sl], start=True, stop=False)
                        nc.tensor.matmul(
                            phi[:, 512 * e:512 * (e + 1)],
                            vt[:, 512 + 128 * jt:512 + 128 * (jt + 1)],
                            mt1[:, sl], start=False, stop=True)
                    sq = sqp.tile([128, 1024], F16, tag=f"sq{jp}")
                    nc.scalar.activation(sq[:], phi[:], AFT.Square,
                                         bias=0.0)
                    sqs.append(sq)
                # probs256: Re^2 + Im^2, k in [0,128) then [128,256)
                pls = []
                for h in range(2):
                    pl = prp.tile([128, 512], F16, tag=f"pr{h}")
                    nc.vector.tensor_add(pl[:], sqs[0][:, 512 * h:512 * (h + 1)],
                                         sqs[1][:, 512 * h:512 * (h + 1)])
                    pls.append(pl)
                return g, pls

            def block_preh(st):
                g, pls = st
                preh = pools['prehp'].tile([LATENT, 512], F32, tag="preh")
                nc.tensor.matmul(preh[:], at[:, 0:4], pls[0][:],
                                 start=True, stop=False)
                nc.tensor.matmul(preh[:], at[:, 4:8], pls[1][:],
                                 start=False, stop=True)
                h5 = sml.tile([LATENT, 512], F16, tag="h5")
                if g % 2 == 0:
                    nc.scalar.activation(h5[:], preh[:], AFT.Relu,
                                         bias=b1s, scale=1.0)
                else:
                    nc.vector.tensor_scalar(h5[:], preh[:], b1s, 0.0,
                                            ALU.add, ALU.max)
                return g, h5

            def block_tail(st, onat):
                g, h5 = st
                wout = pools['woutp'].tile([128, 4 * INPUT_DIM], F32,
                                           tag="wout")
                for c in range(4):
                    nc.tensor.matmul(
                        wout[:, 8 * c:8 * (c + 1)],
                        h5[:, 128 * c:128 * (c + 1)], w2s,
                        start=True, stop=True)
                nc.vector.scalar_tensor_tensor(
                    onat[:, 32 * g:32 * (g + 1)], wout[:], 1.0, b2s,
                    ALU.mult, ALU.add)

            # per-u cos/sin (ACT) + transposes (PE) + copies (DVE) into csT
            with tc.tile_pool(name="tps", bufs=2, space="PSUM") as tpsp:
                for u in range(4):
                    nc.scalar.activation(
                        cnat[:, 128 * u:128 * (u + 1)].rearrange(
                            "p (d n) -> p d n", d=2),
                        xdn[:, 2 * u:2 * u + 2, :], AFT.Sin, scale=0.5,
                        bias=halfpi[:])
                    nc.scalar.activation(
                        snat[:, 128 * u:128 * (u + 1)].rearrange(
                            "p (d n) -> p d n", d=2),
                        xdn[:, 2 * u:2 * u + 2, :], AFT.Sin, scale=0.5,
                        bias=0.0)
                    ctp = tpsp.tile([128, 128], F16, tag="tp")
                    nc.tensor.transpose(ctp[:], cnat[:, 128 * u:128 * (u + 1)],
                                        ids)
                    nc.vector.tensor_copy(csT[:, 128 * u:128 * (u + 1)], ctp[:])
                    stp = tpsp.tile([128, 128], F16, tag="tp")
                    nc.tensor.transpose(stp[:], snat[:, 128 * u:128 * (u + 1)],
                                        ids)
                    nc.vector.tensor_copy(csT[:, 512 + 128 * u:640 + 128 * u],
                                          stp[:])

            with (
                tc.tile_pool(name="phip", bufs=2, space="PSUM") as phip_,
                tc.tile_pool(name="prehp", bufs=1, space="PSUM") as prehp_,
                tc.tile_pool(name="woutp", bufs=1, space="PSUM") as woutp_,
                tc.tile_pool(name="mtap", bufs=1, space="PSUM") as mtap_,
            ):
                pools['phip'] = phip_
                pools['prehp'] = prehp_
                pools['woutp'] = woutp_
                pools['mtap'] = mtap_
                csf_gather()
                # prologue: fully stage phase 0; issue pairsA for 1 and 2
                s0 = stage_q(0)
                s_early = {1: stage_q(1), 2: stage_q(2)}
                m0a = stage_mul1(0, s0[0], s0[1])
                # PE warmup to ramp the clock while staging runs
                for _ in range(NWARM):
                    wrm = phip_.tile([128, 1024], F32, tag="phi")
                    nc.tensor.matmul(wrm[:, 0:512], vt[:, 0:128],
                                     vt[:, 0:512], start=True, stop=True)
                m0b = stage_mul2(0, m0a[1], m0a[2])
                mt = [None] * NPHASE
                a0 = sel_mtA(m0b[0], selA0)
                t0 = stage_mt(a0, m0b[1], "mt0")
                a1 = sel_mtA(m0b[0], selA1)
                mt[0] = (t0, stage_mt(a1, m0b[1], "mt1"))
                s_pend = s_early

                for p in range(NPHASE):
                    onat = sml.tile([128, 8 * NCH], F32, tag="onat")
                    nxt = p + 1
                    stage = nxt < NPHASE
                    if stage:
                        sq_ = s_pend[nxt]
                        mm1 = stage_mul1(nxt, sq_[0], sq_[1])
                    st0 = block_phi(0, mt[p][0], mt[p][1])
                    st1 = block_phi(1, mt[p][0], mt[p][1])
                    if stage:
                        hilo_n, mtB_n = stage_mul2(nxt, mm1[1], mm1[2])
                    st0 = block_preh(st0)
                    if stage:
                        aa0 = sel_mtA(hilo_n, selA0)
                        tt0 = stage_mt(aa0, mtB_n, "mt0")
                    block_tail(st0, onat)
                    st1 = block_preh(st1)
                    if stage:
                        aa1 = sel_mtA(hilo_n, selA1)
                        mt[nxt] = (tt0, stage_mt(aa1, mtB_n, "mt1"))
                        if nxt + 2 < NPHASE:
                            s_pend[nxt + 2] = stage_q(nxt + 2)
                    block_tail(st1, onat)
                    n0 = NCH * p
                    nc.sync.dma_start(
                        out.rearrange("(p n) d -> p n d", n=64)
                        [:, n0:n0 + NCH, :], onat[:])

    nc.compile()
    return nc


_NC_CACHE = []


def _get_nc():
    if not _NC_CACHE:
        _NC_CACHE.append(_build_nc())
    return _NC_CACHE[0]


def kernel(x, q_weights, w1, b1, w2, b2):
    global LAST_RESULTS
    x = np.ascontiguousarray(np.asarray(x, dtype=np.float32))
    consts = _host_consts(np.asarray(q_weights), np.asarray(w1),
                          np.asarray(b1), np.asarray(w2), np.asarray(b2))
    nc = _get_nc()
    in_maps = [
        {'xs': np.ascontiguousarray(x[i * BC:(i + 1) * BC]), **consts}
        for i in range(NCORES)
    ]
    res = run_bass_kernel_spmd(nc, in_maps, list(range(NCORES)))
    LAST_RESULTS = res
    return np.concatenate([res.results[i]['out'] for i in range(NCORES)],
                          axis=0).astype(np.float32)
